# revision 1
# baseline (speedup 1.0000x reference)
"""Causal multi-head attention (dense transformer block) on 8 Trainium2 cores.

Problem: x[4, 2048, 1024], 16 heads, head_dim 64, causal softmax attention
with QKV + output projections (torch Linear layout weights).

Sharding: 8 cores = 4 batches x 2 head-groups (8 heads each).  Each core
computes QKV projection for its 8 heads, attention, and its partial output
projection (row-parallel over w_out).  Host sums the two partials per batch
and adds b_out.

All device layouts are "transposed" so no on-device transposes are needed:
  - x is fed as xT [d, s]; Q^T/K^T are produced as [head_dim, s]
  - scores are computed as S^T [k, q]; softmax denominators come for free
    from a 65th all-ones column appended to each head's V block, so the AV
    matmul emits O^T rows 0..63 and the denominator at row 64 of the same
    PSUM bank in a single rhs stream (no separate ones-matmuls).
  - normalization: the raw denominator row is copied to SBUF, a one-hot
    matmul broadcasts it into rows 64..127 of the same bank, and a
    64-partition DVE copy + reciprocal + multiply produce the normalized
    O^T (engine-batched across the four (hp, h2) chains of a pair).
Matmul inputs are bf16 (PSUM accumulation is fp32); everything else fp32.
"""

import sys

sys.path.insert(0, "/opt/trn_rl_repo")

import numpy as np
import ml_dtypes

import concourse.bass as bass
import concourse.mybir as mybir
import concourse.tile as tile
from concourse import bacc
from concourse import bass_utils
from concourse.masks import make_upper_triangular

F32 = mybir.dt.float32
BF16 = mybir.dt.bfloat16
EXP = mybir.ActivationFunctionType.Exp

B, S, D = 4, 2048, 1024
HTOT, HD = 16, 64
NCORES = 8
HLOC = HTOT // 2          # heads per core
ELOC = HLOC * HD          # 512 local embedding width
NHP = HLOC // 2           # 4 head pairs
QC = 512                  # q-chunk width
NQC = S // QC             # 4
NKT = S // 128            # 16 k tiles over sequence
NDT = D // 128            # 8 k tiles over model dim
SCALE = 1.0 / float(np.sqrt(HD))

_CACHE = {}


def _build_nc():
    nc = bacc.Bacc("TRN2", target_bir_lowering=False, debug=False)

    xT = nc.dram_tensor("xT", [D, S], BF16, kind="ExternalInput")
    wqT = nc.dram_tensor("wqT", [D, ELOC], BF16, kind="ExternalInput")
    wkT = nc.dram_tensor("wkT", [D, ELOC], BF16, kind="ExternalInput")
    wvT = nc.dram_tensor("wvT", [D, ELOC], BF16, kind="ExternalInput")
    woT = nc.dram_tensor("woT", [ELOC, D], BF16, kind="ExternalInput")
    bqk = nc.dram_tensor("bqk", [128, 2, NHP], F32, kind="ExternalInput")
    bvb = nc.dram_tensor("bvb", [128, HLOC, HD], F32, kind="ExternalInput")
    outp = nc.dram_tensor("outp", [S, D], F32, kind="ExternalOutput")

    with tile.TileContext(nc) as tc:
        with tc.tile_pool(name="const", bufs=1) as constp, \
             tc.tile_pool(name="wpool", bufs=1) as wp, \
             tc.tile_pool(name="qkv", bufs=1) as qkvp, \
             tc.tile_pool(name="xt", bufs=1) as xtp, \
             tc.tile_pool(name="pt", bufs=8) as ptp, \
             tc.tile_pool(name="otn", bufs=8) as otnp, \
             tc.tile_pool(name="dr", bufs=4) as drp, \
             tc.tile_pool(name="osb", bufs=6) as osbp:

            # ---- constants ----
            trimask = constp.tile([128, 128], BF16, name="trimask")
            make_upper_triangular(nc, trimask[:], val=1.0, diag=True)
            colones = constp.tile([128, 64], BF16, name="colones")
            nc.gpsimd.memset(colones[:], 0.0)
            nc.gpsimd.memset(colones[0:1, :], 1.0)
            denbank4 = []
            for r in range(4):
                dbt = constp.tile([128, QC], BF16, name=f"denbank{r}")
                nc.gpsimd.memset(dbt[:], 1.0)
                denbank4.append(dbt)

            # ---- weights + xT (order matters: V inputs first so compute
            # starts early; x is streamed in s-chunks, interleaved with the
            # wv tiles, so the first V tiles can begin before the whole of
            # x has landed) ----
            wv_sb = [wp.tile([128, ELOC], BF16, name=f"wv{kt}")
                     for kt in range(NDT)]
            xts = [xtp.tile([128, S], BF16, name=f"xt{kt}")
                   for kt in range(NDT)]
            for kt in range(NDT):
                nc.sync.dma_start(wv_sb[kt][:],
                                  wvT[128 * kt:128 * (kt + 1), :])
                nc.sync.dma_start(
                    xts[kt][:, 0:512], xT[128 * kt:128 * (kt + 1), 0:512])
            bqk_sb = constp.tile([128, 2, NHP], F32, name="bqk_sb")
            nc.sync.dma_start(bqk_sb[:], bqk[:])
            bvb_sb = constp.tile([128, HLOC, HD], F32, name="bvb_sb")
            nc.sync.dma_start(bvb_sb[:], bvb[:])
            for c in range(1, 4):
                for kt in range(NDT):
                    nc.sync.dma_start(
                        xts[kt][:, 512 * c:512 * (c + 1)],
                        xT[128 * kt:128 * (kt + 1), 512 * c:512 * (c + 1)])
            wq_sb, wk_sb = [], []
            for kt in range(NDT):
                for lst, srct, nm in ((wq_sb, wqT, "wq"), (wk_sb, wkT, "wk")):
                    t = wp.tile([128, ELOC], BF16, name=f"{nm}{kt}")
                    nc.sync.dma_start(t[:], srct[128 * kt:128 * (kt + 1), :])
                    lst.append(t)
            wo_sb = []
            for hp in range(NHP):
                t = wp.tile([128, D], BF16, name=f"wo{hp}")
                nc.sync.dma_start(t[:], woT[128 * hp:128 * (hp + 1), :])
                wo_sb.append(t)

            # ---- QKV projection ----
            QT, KT = [], []
            for hp in range(NHP):
                QT.append(qkvp.tile([128, S], BF16, name=f"qt{hp}"))
                KT.append(qkvp.tile([128, S], BF16, name=f"kt{hp}"))
            # V with a 65th all-ones column per head (denominator trick)
            V = [qkvp.tile([128, HLOC, HD + 1], BF16, name=f"v{st}")
                 for st in range(NKT)]
            for st in range(NKT):
                nc.gpsimd.memset(V[st][:, :, HD:HD + 1], 1.0)

            def make_v(psq, st):
                ps = psq.tile([128, HLOC, HD], F32)
                for kt in range(NDT):
                    nc.tensor.matmul(
                        ps[:, :, :],
                        lhsT=xts[kt][:, 128 * st:128 * (st + 1)],
                        rhs=wv_sb[kt][:],
                        start=(kt == 0), stop=(kt == NDT - 1))
                nc.vector.tensor_add(V[st][:, :, 0:HD], ps[:, :, :],
                                     bvb_sb[:, :, :])

            def make_qtkt(psq, hp, tag=""):
                for dst, wsb, col in ((QT, wq_sb, 0), (KT, wk_sb, 1)):
                    for c in range(NQC):
                        ps = psq.tile([128, QC], F32, tag=tag, name="ps")
                        for kt in range(NDT):
                            nc.tensor.matmul(
                                ps[:],
                                lhsT=wsb[kt][:, 128 * hp:128 * (hp + 1)],
                                rhs=xts[kt][:, QC * c:QC * (c + 1)],
                                start=(kt == 0), stop=(kt == NDT - 1))
                        nc.vector.tensor_scalar_add(
                            dst[hp][:, QC * c:QC * (c + 1)], ps[:],
                            bqk_sb[:, col, hp:hp + 1])

            with tc.tile_pool(name="psq", bufs=6, space="PSUM") as psq:
                for st in range(NKT):
                    make_v(psq, st)
                for hp in range(NHP):
                    make_qtkt(psq, hp)

            # ---- attention + output projection ----
            # Two head-pairs run concurrently (PE always has an independent
            # chain while ACT computes exp for the other).  q-chunks go in
            # descending order so the dense j=3 chunk warms the PE clock;
            # the deferred QT/KT (hp 2,3) matmuls fill the attention ramp-up.
            # Per (hp, h2) one PSUM bank holds O^T rows 0..63, the softmax
            # denominator at row 64, and (later) its broadcast reciprocal in
            # rows 64..127.  The same 4-bank ring also serves the output
            # projection and the deferred QT/KT accumulations.
            tri3 = trimask[:][:, None, :].broadcast_to([128, 2, 128])
            with tc.tile_pool(name="pss", bufs=2, space="PSUM") as pss, \
                 tc.tile_pool(name="pso", bufs=4, space="PSUM") as pso:
                for j in (0, 1, 2, 3):
                    nkt = 4 * j + 4
                    otn_j = {}
                    for pair in range(2):
                        hps = (2 * pair, 2 * pair + 1)
                        ps_t = {(hp, h2): pso.tile([128, QC], F32, tag="pso",
                                                   name=f"ps_t{hp}_{h2}")
                                for hp in hps for h2 in range(2)}
                        # Software-pipelined i-loop: AVs trail the scores by
                        # SKEW iterations so a pair-transition AV stall (its
                        # O^T bank waits the previous pair's normalization)
                        # does not head-block the scores/exp stream in the PE
                        # FIFO -- the scalar engine stays fed through the
                        # transition.
                        SKEW = 2
                        pts_hist = {}

                        def emit_avs(iv):
                            wv_ = 128 * (iv - 4 * j) if iv >= 4 * j else 0
                            for hp in hps:
                                pt = pts_hist[iv][hp]
                                for h2 in range(2):
                                    nc.tensor.matmul(
                                        ps_t[hp, h2][0:HD + 1, wv_:QC],
                                        lhsT=V[iv][:, 2 * hp + h2, :],
                                        rhs=pt[:, h2, wv_:QC],
                                        start=(iv == 0),
                                        stop=(iv == nkt - 1))
                            del pts_hist[iv]

                        for i in range(nkt):
                            w = 128 * (i - 4 * j) if i >= 4 * j else 0
                            # scores for both hps grouped (64x128 row tiles)
                            ps_ss = {}
                            for hp in hps:
                                ps_s = pss.tile([128, 2, QC], F32, tag="pss",
                                                name="ps_s")
                                for h2 in range(2):
                                    nc.tensor.matmul(
                                        ps_s[:, h2, w:QC],
                                        lhsT=KT[hp][64 * h2:64 * (h2 + 1),
                                                    128 * i:128 * (i + 1)],
                                        rhs=QT[hp][64 * h2:64 * (h2 + 1),
                                                   QC * j + w:QC * (j + 1)],
                                        start=True, stop=True)
                                ps_ss[hp] = ps_s
                            pts = {}
                            for hp in hps:
                                pt = ptp.tile([128, 2, QC], BF16, tag="pt",
                                              name="pt")
                                nc.scalar.activation(pt[:, :, w:QC],
                                                     ps_ss[hp][:, :, w:QC],
                                                     EXP, scale=SCALE)
                                if i >= 4 * j:
                                    nc.vector.tensor_mul(
                                        pt[:, :, w:w + 128],
                                        pt[:, :, w:w + 128], tri3[:, :, :])
                                pts[hp] = pt
                            pts_hist[i] = pts
                            if i >= SKEW:
                                emit_avs(i - SKEW)
                        for iv in range(max(0, nkt - SKEW), nkt):
                            emit_avs(iv)
                        # normalization: recip the denominator row, K=1
                        # matmul broadcasts it into rows 64..127 of the same
                        # bank, then one 64-row DVE multiply per (hp, h2).
                        # engine-batched normalization: all denominator
                        # copies first, then all broadcast matmuls, then the
                        # DVE tail -- avoids DVE-FIFO head blocking.
                        quads = [(2 * ih + h2, hp, h2, ps_t[hp, h2])
                                 for ih, hp in enumerate(hps)
                                 for h2 in range(2)]
                        for hp in hps:
                            otn_j[hp] = otnp.tile([128, QC], BF16, tag="otn",
                                                  name="otn")
                        for r, hp, h2, t in quads:
                            with nc.allow_low_precision(reason="denom"):
                                nc.vector.tensor_copy(denbank4[r][0:1, :],
                                                      t[HD:HD + 1, :])
                        for r, hp, h2, t in quads:
                            nc.tensor.matmul(
                                t[64:128, :], lhsT=colones[:],
                                rhs=denbank4[r][:], start=True, stop=True,
                                tile_position=(0, 64))
                        rdbcs = {}
                        for r, hp, h2, t in quads:
                            rdbc_f = drp.tile([64, QC], F32, name="rdbcf",
                                              bufs=8)
                            nc.vector.tensor_copy(rdbc_f[:], t[64:128, :])
                            rdbcs[r] = rdbc_f
                        for r, hp, h2, t in quads:
                            rdbc = drp.tile([64, QC], F32, name="rdbc",
                                            tag="rdbc", bufs=8)
                            nc.vector.reciprocal_approx_fast(rdbc[:],
                                                             rdbcs[r][:])
                            rdbcs[r] = rdbc
                        for r, hp, h2, t in quads:
                            nc.vector.tensor_mul(
                                otn_j[hp][64 * h2:64 * (h2 + 1), :],
                                t[0:64, :], rdbcs[r][:])
                    # output projection for this q chunk
                    for m in range(4):
                        s0 = QC * j + 128 * m
                        for eo in range(2):
                            ps_o = pso.tile([128, 512], F32, tag="pso",
                                            name="ps_o")
                            for hp in range(NHP):
                                nc.tensor.matmul(
                                    ps_o[:],
                                    lhsT=otn_j[hp][:, 128 * m:128 * (m + 1)],
                                    rhs=wo_sb[hp][:, 512 * eo:512 * (eo + 1)],
                                    start=(hp == 0), stop=(hp == NHP - 1))
                            osb = osbp.tile([128, 512], F32)
                            nc.vector.tensor_copy(osb[:], ps_o[:])
                            nc.sync.dma_start(
                                outp[s0:s0 + 128, 512 * eo:512 * (eo + 1)],
                                osb[:])
    nc.compile()
    return nc


def _get_nc():
    if "nc" not in _CACHE:
        _CACHE["nc"] = _build_nc()
    return _CACHE["nc"]


def _prep_core_inputs(x, w_qkv, b_qkv, w_out, b, hg):
    r0 = ELOC * hg
    wq = w_qkv[r0:r0 + ELOC, :]
    wk = w_qkv[D + r0:D + r0 + ELOC, :]
    wv = w_qkv[2 * D + r0:2 * D + r0 + ELOC, :]
    bq = b_qkv[r0:r0 + ELOC]
    bk = b_qkv[D + r0:D + r0 + ELOC]
    bv = b_qkv[2 * D + r0:2 * D + r0 + ELOC]

    bf = ml_dtypes.bfloat16
    bqk_arr = np.empty((128, 2, NHP), np.float32)
    bqk_arr[:, 0, :] = bq.reshape(NHP, 128).T
    bqk_arr[:, 1, :] = bk.reshape(NHP, 128).T
    return {
        "xT": np.ascontiguousarray(x[b].T).astype(bf),
        "wqT": np.ascontiguousarray(wq.T).astype(bf),
        "wkT": np.ascontiguousarray(wk.T).astype(bf),
        "wvT": np.ascontiguousarray(wv.T).astype(bf),
        "woT": np.ascontiguousarray(w_out[:, r0:r0 + ELOC].T).astype(bf),
        "bqk": bqk_arr,
        "bvb": np.tile(bv.astype(np.float32)[None, :],
                       (128, 1)).reshape(128, HLOC, HD),
    }


def kernel(x, w_qkv, b_qkv, w_out, b_out, _trace=False, _trace_kwargs=None):
    x = np.asarray(x, np.float32)
    w_qkv = np.asarray(w_qkv, np.float32)
    b_qkv = np.asarray(b_qkv, np.float32)
    w_out = np.asarray(w_out, np.float32)
    b_out = np.asarray(b_out, np.float32)

    nc = _get_nc()
    in_maps = []
    for core in range(NCORES):
        b, hg = core // 2, core % 2
        in_maps.append(_prep_core_inputs(x, w_qkv, b_qkv, w_out, b, hg))

    kw = {}
    if _trace:
        kw.update(trace=True, **(_trace_kwargs or {}))
    import time
    res = None
    for attempt in range(4):
        try:
            res = bass_utils.run_bass_kernel_spmd(
                nc, in_maps, core_ids=list(range(NCORES)), **kw)
            break
        except Exception:
            if attempt == 3:
                raise
            # Transient axon/NRT device flake: reset the PJRT backend so the
            # retry starts from a clean client, like a fresh process would.
            try:
                import jax
                jax.clear_caches()
                import jax._src.xla_bridge as _xb
                _xb._clear_backends()
            except Exception:
                pass
            time.sleep(5.0 * (attempt + 1))

    out = np.empty((B, S, D), np.float32)
    for b in range(B):
        out[b] = res.results[2 * b]["outp"] + res.results[2 * b + 1]["outp"] \
            + b_out[None, :]
    if _trace:
        return out, res
    return out



# revision 8
# speedup vs baseline: 1.0992x; 1.0992x over previous
"""Causal multi-head attention (dense transformer block) on 8 Trainium2 cores.

Problem: x[4, 2048, 1024], 16 heads, head_dim 64, causal softmax attention
with QKV + output projections (torch Linear layout weights).

Sharding: 8 cores = 4 batches x 2 head-groups (8 heads each).  Each core
computes QKV projection for its 8 heads, attention, and its partial output
projection (row-parallel over w_out).  Host sums the two partials per batch
and adds b_out.

Device layouts are "transposed" so no on-device transposes are needed:
  - x is fed as xT [d, s]; Q^T/K^T are produced as [head_dim, s]
  - scores are computed as S^T [k, q]; the two heads of a pair run as
    row-group-tiled concurrent matmuls (K=64 contraction at array rows
    0-63 / 64-127).
  - AV is col-group packed: per head-pair one PSUM bank holds O^T for
    head A in partitions 0..63 and head B in partitions 64..127, written
    by two concurrent col-tiled matmuls (tile_position auto-derived).
  - softmax denominators come from four col-packed M=1 ones-matmuls per
    i-step accumulating into rows 0/32/64/96 of a dedicated PSUM bank.
  - normalization: denominator rows are copied (partition-remapped) to
    SBUF, reciprocal'd at [2,512] cost, broadcast into a full 128-row
    PSUM bank by one K=2 selector matmul per hp, copied to SBUF, and
    applied with one [128,512] DVE multiply per hp.
  - PE filler: the deferred Q-chunk projections (j>=1), deferred V tiles
    (st>=12), and the output projections are emitted *between* attention
    i-steps so the tensor engine never idles while the scalar engine
    (exp) catches up -- this also keeps the PE HAM clock un-throttled.
Matmul inputs are bf16 (PSUM accumulation is fp32); everything else fp32.
"""

import sys

sys.path.insert(0, "/opt/trn_rl_repo")

from collections import deque

import numpy as np
import ml_dtypes

import concourse.bass as bass
import concourse.mybir as mybir
import concourse.tile as tile
from concourse import bacc
from concourse import bass_utils
from concourse.masks import make_upper_triangular

F32 = mybir.dt.float32
BF16 = mybir.dt.bfloat16
EXP = mybir.ActivationFunctionType.Exp

B, S, D = 4, 2048, 1024
HTOT, HD = 16, 64
NCORES = 8
HLOC = HTOT // 2          # heads per core
ELOC = HLOC * HD          # 512 local embedding width
NHP = HLOC // 2           # 4 head pairs
QC = 512                  # q-chunk width
NQC = S // QC             # 4
NKT = S // 128            # 16 k tiles over sequence
NDT = D // 128            # 8 k tiles over model dim
SCALE = 1.0 / float(np.sqrt(HD))
NVUP = 12                 # V s-tiles computed upfront; the rest are filler

_CACHE = {}


def _build_nc():
    nc = bacc.Bacc("TRN2", target_bir_lowering=False, debug=False)

    xT = nc.dram_tensor("xT", [D, S], BF16, kind="ExternalInput")
    wqT = nc.dram_tensor("wqT", [D, ELOC], BF16, kind="ExternalInput")
    wkT = nc.dram_tensor("wkT", [D, ELOC], BF16, kind="ExternalInput")
    wvT = nc.dram_tensor("wvT", [D, ELOC], BF16, kind="ExternalInput")
    woT = nc.dram_tensor("woT", [ELOC, D], BF16, kind="ExternalInput")
    bqk = nc.dram_tensor("bqk", [128, 2, NHP], F32, kind="ExternalInput")
    bvb = nc.dram_tensor("bvb", [128, HLOC, HD], F32, kind="ExternalInput")
    outp = nc.dram_tensor("outp", [S, D], F32, kind="ExternalOutput")

    with tile.TileContext(nc) as tc:
        with tc.tile_pool(name="const", bufs=1) as constp, \
             tc.tile_pool(name="wpool", bufs=1) as wp, \
             tc.tile_pool(name="qkv", bufs=1) as qkvp, \
             tc.tile_pool(name="xt", bufs=1) as xtp, \
             tc.tile_pool(name="pt", bufs=8) as ptp, \
             tc.tile_pool(name="otn", bufs=8) as otnp, \
             tc.tile_pool(name="dr", bufs=8) as drp, \
             tc.tile_pool(name="rds", bufs=2) as rdsp, \
             tc.tile_pool(name="osb", bufs=4) as osbp:

            # ---- constants ----
            trimask = constp.tile([128, 128], BF16, name="trimask")
            make_upper_triangular(nc, trimask[:], val=1.0, diag=True)
            ones1 = constp.tile([128, 1], BF16, name="ones1")
            nc.gpsimd.memset(ones1[:], 1.0)
            # ones row for the K=1 denominator-broadcast matmuls
            onesr = constp.tile([1, 64], BF16, name="onesr")
            nc.gpsimd.memset(onesr[:], 1.0)

            # ---- weights + xT (V inputs first so compute starts early;
            # x is streamed in s-chunks interleaved with the wv tiles) ----
            wv_sb = [wp.tile([128, ELOC], BF16, name=f"wv{kt}")
                     for kt in range(NDT)]
            xts = [xtp.tile([128, S], BF16, name=f"xt{kt}")
                   for kt in range(NDT)]
            for kt in range(NDT):
                nc.sync.dma_start(wv_sb[kt][:],
                                  wvT[128 * kt:128 * (kt + 1), :])
                nc.sync.dma_start(
                    xts[kt][:, 0:512], xT[128 * kt:128 * (kt + 1), 0:512])
            bqk_sb = constp.tile([128, 2, NHP], F32, name="bqk_sb")
            nc.sync.dma_start(bqk_sb[:], bqk[:])
            bvb_sb = constp.tile([128, HLOC, HD], F32, name="bvb_sb")
            nc.sync.dma_start(bvb_sb[:], bvb[:])
            for c in range(1, 4):
                for kt in range(NDT):
                    nc.sync.dma_start(
                        xts[kt][:, 512 * c:512 * (c + 1)],
                        xT[128 * kt:128 * (kt + 1), 512 * c:512 * (c + 1)])
            wq_sb, wk_sb = [], []
            for kt in range(NDT):
                for lst, srct, nm in ((wq_sb, wqT, "wq"), (wk_sb, wkT, "wk")):
                    t = wp.tile([128, ELOC], BF16, name=f"{nm}{kt}")
                    nc.sync.dma_start(t[:], srct[128 * kt:128 * (kt + 1), :])
                    lst.append(t)
            wo_sb = []
            for hp in range(NHP):
                t = wp.tile([128, D], BF16, name=f"wo{hp}")
                nc.sync.dma_start(t[:], woT[128 * hp:128 * (hp + 1), :])
                wo_sb.append(t)

            # ---- QKV projection tiles ----
            QT, KT = [], []
            for hp in range(NHP):
                QT.append(qkvp.tile([128, S], BF16, name=f"qt{hp}"))
                KT.append(qkvp.tile([128, S], BF16, name=f"kt{hp}"))
            V = [qkvp.tile([128, HLOC, HD], BF16, name=f"v{st}")
                 for st in range(NKT)]

            def make_v(pool, st, tag=""):
                ps = pool.tile([128, HLOC, HD], F32, tag=tag or "psA", name="psv")
                for kt in range(NDT):
                    nc.tensor.matmul(
                        ps[:, :, :],
                        lhsT=xts[kt][:, 128 * st:128 * (st + 1)],
                        rhs=wv_sb[kt][:],
                        start=(kt == 0), stop=(kt == NDT - 1))
                nc.vector.tensor_add(V[st][:, :, :], ps[:, :, :],
                                     bvb_sb[:, :, :])

            def make_proj(pool, dst, wsb, col, hp, c, tag=""):
                # one 512-wide chunk of Q^T or K^T for head-pair hp
                ps = pool.tile([128, QC], F32, tag=tag or "psA", name="psp")
                for kt in range(NDT):
                    nc.tensor.matmul(
                        ps[:],
                        lhsT=wsb[kt][:, 128 * hp:128 * (hp + 1)],
                        rhs=xts[kt][:, QC * c:QC * (c + 1)],
                        start=(kt == 0), stop=(kt == NDT - 1))
                nc.vector.tensor_scalar_add(
                    dst[hp][:, QC * c:QC * (c + 1)], ps[:],
                    bqk_sb[:, col, hp:hp + 1])

            # ---- phase A: V (first NVUP tiles), all K^T, Q^T chunk 0 ----
            with tc.tile_pool(name="psq", bufs=6, space="PSUM") as psq:
                for st in range(NVUP):
                    make_v(psq, st)
                for hp in range(NHP):
                    for c in range(NQC):
                        make_proj(psq, KT, wk_sb, 1, hp, c)
                for hp in range(NHP):
                    make_proj(psq, QT, wq_sb, 0, hp, 0)

            # ---- phase B: attention with PE filler ----
            tri3 = trimask[:][:, None, :].broadcast_to([128, 2, 128])
            otn_store = {}
            filler = deque()

            with tc.tile_pool(name="pss", bufs=2, space="PSUM") as pss, \
                 tc.tile_pool(name="pst", bufs=2, space="PSUM") as pstp, \
                 tc.tile_pool(name="dnp", bufs=1, space="PSUM") as dnp, \
                 tc.tile_pool(name="flt", bufs=1, space="PSUM") as flt:

                def qt_unit(hp, c):
                    def go():
                        make_proj(flt, QT, wq_sb, 0, hp, c, tag="flt")
                    return go

                def v_unit(st):
                    def go():
                        make_v(flt, st, tag="flt")
                    return go

                def oproj_unit(j, m, eo):
                    def go():
                        s0 = QC * j + 128 * m
                        ps_o = flt.tile([128, 512], F32, tag="flt",
                                        name="ps_o")
                        for hp in range(NHP):
                            nc.tensor.matmul(
                                ps_o[:],
                                lhsT=otn_store[(j, hp)][:,
                                                        128 * m:128 * (m + 1)],
                                rhs=wo_sb[hp][:, 512 * eo:512 * (eo + 1)],
                                start=(hp == 0), stop=(hp == NHP - 1))
                        osb = osbp.tile([128, 512], F32)
                        nc.vector.tensor_copy(osb[:], ps_o[:])
                        nc.sync.dma_start(
                            outp[s0:s0 + 128, 512 * eo:512 * (eo + 1)],
                            osb[:])
                    return go

                def pump(n):
                    for _ in range(n):
                        if not filler:
                            return
                        filler.popleft()()

                def chain(j, pair):
                    nkt = 4 * j + 4
                    hps = (2 * pair, 2 * pair + 1)
                    ps_t = {hp: pstp.tile([128, QC], F32, tag="pst",
                                          name=f"ps_t{hp}") for hp in hps}
                    denps = dnp.tile([128, QC], F32, tag="dnp", name="denps")
                    SKEW = 2
                    pts_hist = {}

                    def emit_avs(iv):
                        wv_ = 128 * (iv - 4 * j) if iv >= 4 * j else 0
                        for hp in hps:
                            pt = pts_hist[iv][hp]
                            for h2 in range(2):
                                nc.tensor.matmul(
                                    ps_t[hp][64 * h2:64 * (h2 + 1), wv_:QC],
                                    lhsT=V[iv][:, 2 * hp + h2, :],
                                    rhs=pt[:, h2, wv_:QC],
                                    start=(iv == 0),
                                    stop=(iv == nkt - 1))
                        for qi, (hp, h2) in enumerate(
                                (hp, h2) for hp in hps for h2 in range(2)):
                            pt = pts_hist[iv][hp]
                            nc.tensor.matmul(
                                denps[32 * qi:32 * qi + 1, wv_:QC],
                                lhsT=ones1[:, 0:1],
                                rhs=pt[:, h2, wv_:QC],
                                start=(iv == 0),
                                stop=(iv == nkt - 1),
                                tile_position=(0, 32 * qi))
                        del pts_hist[iv]

                    for i in range(nkt):
                        w = 128 * (i - 4 * j) if i >= 4 * j else 0
                        ps_ss = {}
                        for hp in hps:
                            ps_s = pss.tile([128, 2, QC], F32, tag="pss",
                                            name="ps_s")
                            for h2 in range(2):
                                nc.tensor.matmul(
                                    ps_s[:, h2, w:QC],
                                    lhsT=KT[hp][64 * h2:64 * (h2 + 1),
                                                128 * i:128 * (i + 1)],
                                    rhs=QT[hp][64 * h2:64 * (h2 + 1),
                                               QC * j + w:QC * (j + 1)],
                                    start=True, stop=True)
                            ps_ss[hp] = ps_s
                        pts = {}
                        for hp in hps:
                            pt = ptp.tile([128, 2, QC], BF16, tag="pt",
                                          name="pt")
                            nc.scalar.activation(pt[:, :, w:QC],
                                                 ps_ss[hp][:, :, w:QC],
                                                 EXP, scale=SCALE)
                            if i >= 4 * j:
                                nc.vector.tensor_mul(
                                    pt[:, :, w:w + 128],
                                    pt[:, :, w:w + 128], tri3[:, :, :])
                            pts[hp] = pt
                        pts_hist[i] = pts
                        if i >= SKEW:
                            emit_avs(i - SKEW)
                        pump(1)
                    for iv in range(max(0, nkt - SKEW), nkt):
                        emit_avs(iv)

                    # ---- normalization ----
                    # each denominator row -> its own [1,512] SBUF tile at
                    # partition 0 (32-aligned partition remap only), then
                    # reciprocal + bf16 cast, then one K=1 broadcast matmul
                    # per (hp, h2) into a full [128,512] PSUM bank.
                    quads = [(qi, hp, h2) for qi, (hp, h2) in enumerate(
                        (hp, h2) for hp in hps for h2 in range(2))]
                    den_rb = {}
                    for qi, hp, h2 in quads:
                        dsb = drp.tile([1, QC], F32, name="den_sb",
                                       tag="den")
                        nc.vector.tensor_copy(
                            dsb[:], denps[32 * qi:32 * qi + 1, :])
                        dr = drp.tile([1, QC], F32, name="den_r", tag="den")
                        nc.vector.reciprocal_approx_fast(dr[:], dsb[:])
                        drb = drp.tile([1, QC], BF16, name="den_rb",
                                       tag="den")
                        with nc.allow_low_precision(reason="denom"):
                            nc.vector.tensor_copy(drb[:], dr[:])
                        den_rb[hp, h2] = drb
                    for hp in hps:
                        otn = otnp.tile([128, QC], BF16, tag="otn",
                                        name="otn")
                        rdps = flt.tile([128, QC], F32, tag="flt",
                                        name="rdps")
                        for h2 in range(2):
                            nc.tensor.matmul(
                                rdps[64 * h2:64 * (h2 + 1), :],
                                lhsT=onesr[:],
                                rhs=den_rb[hp, h2][:],
                                start=True, stop=True)
                        rdsb = rdsp.tile([128, QC], F32, name="rdsb")
                        nc.vector.tensor_copy(rdsb[:], rdps[:])
                        nc.vector.tensor_mul(otn[:], ps_t[hp][:], rdsb[:])
                        otn_store[(j, hp)] = otn

                for j in range(NQC):
                    if j + 1 < NQC:
                        for hp in range(NHP):
                            filler.append(qt_unit(hp, j + 1))
                    if j == 3:
                        for st in range(NVUP, NKT):
                            filler.append(v_unit(st))
                    chain(j, 0)
                    if j >= 1:
                        for m in range(4):
                            for eo in range(2):
                                filler.append(oproj_unit(j - 1, m, eo))
                    chain(j, 1)
                # drain remaining filler + final output projection
                while filler:
                    filler.popleft()()
                for m in range(4):
                    for eo in range(2):
                        oproj_unit(3, m, eo)()
    nc.compile()
    return nc


def _get_nc():
    if "nc" not in _CACHE:
        _CACHE["nc"] = _build_nc()
    return _CACHE["nc"]


def _prep_core_inputs(x, w_qkv, b_qkv, w_out, b, hg):
    r0 = ELOC * hg
    wq = w_qkv[r0:r0 + ELOC, :]
    wk = w_qkv[D + r0:D + r0 + ELOC, :]
    wv = w_qkv[2 * D + r0:2 * D + r0 + ELOC, :]
    bq = b_qkv[r0:r0 + ELOC]
    bk = b_qkv[D + r0:D + r0 + ELOC]
    bv = b_qkv[2 * D + r0:2 * D + r0 + ELOC]

    bf = ml_dtypes.bfloat16
    bqk_arr = np.empty((128, 2, NHP), np.float32)
    bqk_arr[:, 0, :] = bq.reshape(NHP, 128).T
    bqk_arr[:, 1, :] = bk.reshape(NHP, 128).T
    return {
        "xT": np.ascontiguousarray(x[b].T).astype(bf),
        "wqT": np.ascontiguousarray(wq.T).astype(bf),
        "wkT": np.ascontiguousarray(wk.T).astype(bf),
        "wvT": np.ascontiguousarray(wv.T).astype(bf),
        "woT": np.ascontiguousarray(w_out[:, r0:r0 + ELOC].T).astype(bf),
        "bqk": bqk_arr,
        "bvb": np.tile(bv.astype(np.float32)[None, :],
                       (128, 1)).reshape(128, HLOC, HD),
    }


def kernel(x, w_qkv, b_qkv, w_out, b_out, _trace=False, _trace_kwargs=None):
    x = np.asarray(x, np.float32)
    w_qkv = np.asarray(w_qkv, np.float32)
    b_qkv = np.asarray(b_qkv, np.float32)
    w_out = np.asarray(w_out, np.float32)
    b_out = np.asarray(b_out, np.float32)

    nc = _get_nc()
    in_maps = []
    for core in range(NCORES):
        b, hg = core // 2, core % 2
        in_maps.append(_prep_core_inputs(x, w_qkv, b_qkv, w_out, b, hg))

    kw = {}
    if _trace:
        kw.update(trace=True, **(_trace_kwargs or {}))
    import time
    res = None
    for attempt in range(4):
        try:
            res = bass_utils.run_bass_kernel_spmd(
                nc, in_maps, core_ids=list(range(NCORES)), **kw)
            break
        except Exception:
            if attempt == 3:
                raise
            # Transient axon/NRT device flake: reset the PJRT backend so the
            # retry starts from a clean client, like a fresh process would.
            try:
                import jax
                jax.clear_caches()
                import jax._src.xla_bridge as _xb
                _xb._clear_backends()
            except Exception:
                pass
            time.sleep(5.0 * (attempt + 1))

    out = np.empty((B, S, D), np.float32)
    for b in range(B):
        out[b] = res.results[2 * b]["outp"] + res.results[2 * b + 1]["outp"] \
            + b_out[None, :]
    if _trace:
        return out, res
    return out
